# revision 30
# baseline (speedup 1.0000x reference)
"""Trainium2 Bass kernel for nn_DetectionHead (VoteNet-style detection head).

Self-contained 8-core SPMD kernel. Strategy:
  - FPS (128 serial argmax iterations) is the critical path. Each iteration
    is a fused Vector/GpSimd chain: tensor_tensor_reduce fuses the min_d
    update with the row-max; the per-row argmax payload (masked sums of
    [-2p, 1, |p|^2]) is computed while GpSimd runs the max all-reduce; the
    add all-reduce then both broadcasts the winner payload AND records it
    (each iteration writes its own 5-column slice of `sgrp`).
  - Clusters are processed in 16 fine groups (1 cluster per core per group,
    cluster m = 8*g + core) overlapped under FPS: per-group d^2 mask via a
    single stacked [5,x]@[5,512] matmul per chunk, fp16 W2 layer, -BIG mask
    injection, chunked max-pool, batched box head.
  - One AllGather of the per-core [7,16] box logits, then replicated NMS.

kernel(**inputs) takes the full unsharded inputs, returns the full [128,6]
output.
"""

import numpy as np

NCORES = 8
N = 4096          # points
C = 128           # feature channels
M = 128           # clusters
NJ = 32           # points per partition (N = 128 * NJ)
NG = 16           # cluster groups (1 cluster per core per group)
RADIUS = 0.5
THR = RADIUS * RADIUS
NMS_THR = 0.25
BIGM = 8388608.0  # 2^23, exact in bf16/f32r
NMS_ITERS = 6
CHUNK = 512
NCHUNK = N // CHUNK       # 8

_cache = {}


def _build(debug=False):
    import concourse.bacc as bacc
    import concourse.tile as tile
    import concourse.mybir as mybir
    import concourse.bass_isa as bass_isa

    F32 = mybir.dt.float32
    F32R = mybir.dt.float32r
    F16 = mybir.dt.float16
    BF16 = mybir.dt.bfloat16
    I32 = mybir.dt.int32
    ALU = mybir.AluOpType
    ACTF = mybir.ActivationFunctionType
    AX = mybir.AxisListType
    FLT_MAX = 3.4028234e38

    nc = bacc.Bacc("TRN2", target_bir_lowering=False, debug=False,
                   num_devices=NCORES)

    # ---- DRAM I/O ----
    d_pts96 = nc.dram_tensor("pts96", [128, 96], F32, kind="ExternalInput")
    d_pT = nc.dram_tensor("pT", [3, N], F32, kind="ExternalInput")
    d_featT = nc.dram_tensor("featT", [C, N], F32, kind="ExternalInput")
    d_W1a = nc.dram_tensor("W1a", [3, C], F32, kind="ExternalInput")
    d_W1b = nc.dram_tensor("W1b", [C, C], F32, kind="ExternalInput")
    d_W2 = nc.dram_tensor("W2", [C, C], F32, kind="ExternalInput")
    d_W3 = nc.dram_tensor("W3", [C, C], F32, kind="ExternalInput")
    d_W4 = nc.dram_tensor("W4", [C, C], F32, kind="ExternalInput")
    d_Wf = nc.dram_tensor("Wf", [C, 7], F32, kind="ExternalInput")
    d_b1r = nc.dram_tensor("b1r", [1, C], F32, kind="ExternalInput")
    d_b2c = nc.dram_tensor("b2c", [C, 1], F32, kind="ExternalInput")
    d_b3c = nc.dram_tensor("b3c", [C, 1], F32, kind="ExternalInput")
    d_b4c = nc.dram_tensor("b4c", [C, 1], F32, kind="ExternalInput")
    d_bfr = nc.dram_tensor("bfr", [1, 7], F32, kind="ExternalInput")
    d_sel540 = nc.dram_tensor("sel540", [64, 5], F32, kind="ExternalInput")

    d_out = nc.dram_tensor("out", [M, 6], F32, kind="ExternalOutput")
    if debug:
        d_dbg_sgrp = nc.dram_tensor("dbg_sgrp", [128, 8 * M], F32,
                                    kind="ExternalOutput")
        d_dbg_boxT = nc.dram_tensor("dbg_boxT", [7, NG], F32,
                                    kind="ExternalOutput")
        d_dbg_BT = nc.dram_tensor("dbg_BT", [7, 128], F32,
                                  kind="ExternalOutput")
        d_dbg_keep = nc.dram_tensor("dbg_keep", [128, 1], F32,
                                    kind="ExternalOutput")
        d_dbg_U2b = nc.dram_tensor("dbg_U2b", [128, 1], F32,
                                   kind="ExternalOutput")
        d_dbg_mask = nc.dram_tensor("dbg_mask", [1, N], BF16,
                                    kind="ExternalOutput")
        d_dbg_BX = nc.dram_tensor("dbg_BX", [128, 14], F32,
                                  kind="ExternalOutput")
        d_dbg_Ps = nc.dram_tensor("dbg_Ps", [128, 128], F32,
                                  kind="ExternalOutput")
        d_dbg_G = nc.dram_tensor("dbg_G", [128, 4], F32,
                                 kind="ExternalOutput")
        d_dbg_psk = nc.dram_tensor("dbg_psk", [128, 2], F32,
                                   kind="ExternalOutput")
        d_dbg_keep1 = nc.dram_tensor("dbg_keep1", [128, 2], F32,
                                     kind="ExternalOutput")
        d_dbg_Ps2 = nc.dram_tensor("dbg_Ps2", [128, 128], F32,
                                   kind="ExternalOutput")

    from contextlib import ExitStack
    es = ExitStack()
    with tile.TileContext(nc) as tc:
        cp = es.enter_context(tc.tile_pool(name="const", bufs=1))
        stage_es = ExitStack()
        stage = stage_es.enter_context(tc.tile_pool(name="stage", bufs=1))

        # ---- persistent tiles ----
        pts96 = cp.tile([128, 96], F32)
        pT = cp.tile([3, N], F32)
        featT = stage.tile([C, N], F32R, name="featTr")
        X5kj = cp.tile([128, 5 * NJ], F32)    # k-major: [-2p | 1 | psq]
        P5jk = cp.tile([128, NJ * 5], F32)    # j-major: [p | psq | 1]
        psq = cp.tile([128, NJ], F32)
        min_d = cp.tile([128, NJ], F32)
        junk32 = cp.tile([128, NJ], F32)
        dnm = cp.tile([128, NJ * 5], F32)
        dn = cp.tile([128, NJ], F32)
        rowmax = cp.tile([128, 1], F32)
        gb = cp.tile([128, 1], F32)
        Rm = cp.tile([128, 5 * NJ], F32)
        R5 = cp.tile([128, 5], F32)
        P5s = cp.tile([128, 5], F32)
        sgrp = cp.tile([128, 8 * M], F32)     # 8-col (32B) aligned records
        sel540 = cp.tile([64, 5], F32)        # per-core record extractor
        sb40 = [cp.tile([64, 1], F32, name=f"sb40_{i}") for i in range(2)]
        one1f = cp.tile([1, 1], F32)
        W1ab1 = cp.tile([4, C], F32)          # [W1a; b1]
        W1am2 = cp.tile([3, C], F32R)         # -2 * W1a
        W2h = cp.tile([C, C], F16)
        W3 = cp.tile([C, C], F32R)
        W4 = cp.tile([C, C], F32R)
        Wf = cp.tile([C, 7], F32R)
        b2c = cp.tile([C, 1], F32)
        b3c = cp.tile([C, 1], F32)
        b4c = cp.tile([C, 1], F32)
        bfr = cp.tile([1, 7], F32R)
        ones128b = cp.tile([1, 128], BF16)    # NB-inject stationary
        ones1x4 = cp.tile([1, 4], F32R)
        s5row = [cp.tile([1, 5], F32, name=f"s5row{i}") for i in range(2)]
        s5own = [cp.tile([128, 5], F32, name=f"s5own{i}") for i in range(2)]
        d2m = cp.tile([128, NJ * 5], F32)
        d2g = cp.tile([128, NJ], F32)
        ident = cp.tile([128, 128], F32)
        ident_i = cp.tile([128, 128], I32)
        iotaN_i = cp.tile([128, NJ], I32)
        P3 = cp.tile([C, N], F32)
        ctmo = [cp.tile([4, 1], F32, name=f"ctmo{i}") for i in range(2)]
        mstat = [cp.tile([5, 1], F32, name=f"mstat{i}") for i in range(2)]
        U2bk = [cp.tile([C, 1], F32, name=f"u2bk{i}") for i in range(2)]
        G4col = [cp.tile([128, 4], F32, name=f"g4col{i}") for i in range(2)]
        boxT = cp.tile([7, NG], F32)
        BTall = cp.tile([7, 128], F32)

        # stage tiles (freed after setup)
        tmp96 = stage.tile([128, 96], F32)

        def r_(ap):
            return ap.bitcast(F32R)

        # ---- input DMA ----
        nc.sync.dma_start(pts96[:], d_pts96.ap())
        nc.sync.dma_start(pT[:], d_pT.ap())
        featT_st = stage.tile([C, N], F32, name="featT_st")
        pTr = stage.tile([3, N], F32R, name="pTr")
        W1b_st = stage.tile([C, C], F32, name="w1b_st")
        W1br = stage.tile([C, C], F32R, name="w1br")
        W3_st = stage.tile([C, C], F32, name="w3_st")
        W4_st = stage.tile([C, C], F32, name="w4_st")
        Wf_st = stage.tile([C, 7], F32, name="wf_st")
        bfr_st = stage.tile([1, 7], F32, name="bfr_st")
        nc.sync.dma_start(featT_st[:], d_featT.ap())
        nc.sync.dma_start(W1ab1[0:3, :], d_W1a.ap())
        nc.sync.dma_start(W1ab1[3:4, :], d_b1r.ap())
        nc.sync.dma_start(W1b_st[:], d_W1b.ap())
        W2h_stage = stage.tile([C, C], F32, name="w2stage")
        nc.sync.dma_start(W2h_stage[:], d_W2.ap())
        nc.sync.dma_start(W3_st[:], d_W3.ap())
        nc.sync.dma_start(W4_st[:], d_W4.ap())
        nc.sync.dma_start(Wf_st[:], d_Wf.ap())
        nc.sync.dma_start(b2c[:], d_b2c.ap())
        nc.sync.dma_start(b3c[:], d_b3c.ap())
        nc.sync.dma_start(b4c[:], d_b4c.ap())
        nc.sync.dma_start(bfr_st[:], d_bfr.ap())
        nc.sync.dma_start(sel540[:], d_sel540.ap())

        # ---- constants ----
        nc.gpsimd.iota(ident_i[:], pattern=[[1, 128]], base=0,
                       channel_multiplier=-1)
        nc.vector.tensor_scalar(ident[:], ident_i[:], 0, None,
                                op0=ALU.is_equal)
        nc.gpsimd.iota(iotaN_i[:], pattern=[[1, NJ]], base=0,
                       channel_multiplier=NJ)
        nc.vector.tensor_scalar(min_d[:], iotaN_i[:], -1.0, None,
                                op0=ALU.mult)
        o4f = stage.tile([1, 4], F32, name="o4f")
        ob = stage.tile([1, 128], F32, name="ob")
        nc.vector.memset(o4f[:], 1.0)
        nc.vector.tensor_copy(ones1x4[:], o4f[:])
        nc.vector.memset(ob[:], 1.0)
        nc.vector.tensor_copy(ones128b[:], ob[:])
        nc.vector.memset(one1f[:], 1.0)
        nc.vector.tensor_copy(W2h[:], W2h_stage[:])
        nc.vector.tensor_copy(featT[:], featT_st[:])
        nc.scalar.copy(pTr[:], pT[:])
        nc.vector.tensor_copy(W1br[:], W1b_st[:])
        nc.vector.tensor_copy(W3[:], W3_st[:])
        nc.vector.tensor_copy(W4[:], W4_st[:])
        nc.vector.tensor_copy(Wf[:], Wf_st[:])
        nc.vector.tensor_copy(bfr[:], bfr_st[:])
        nc.scalar.mul(W1am2[:], W1ab1[0:3, :], -2.0)
        for i in range(2):
            nc.vector.memset(ctmo[i][:], 1.0)  # row 3 stays 1.0

        # psq[p, j] = |p_n|^2 for n = 32p + j
        nc.vector.tensor_mul(tmp96[:], pts96[:], pts96[:])
        nc.vector.tensor_reduce(
            psq[:], tmp96[:].rearrange("p (j c) -> p j c", c=3),
            axis=AX.X, op=ALU.add)

        # X5kj: k-major rows [-2px, -2py, -2pz, 1, psq]
        nc.scalar.mul(
            X5kj[:].rearrange("p (k j) -> p k j", j=NJ)[:, 0:3, :],
            pts96[:].rearrange("p (j c) -> p c j", c=3), -2.0)
        nc.vector.memset(X5kj[:, 3 * NJ:4 * NJ], 1.0)
        nc.vector.tensor_copy(X5kj[:, 4 * NJ:5 * NJ], psq[:])

        # P5jk: j-major rows [px, py, pz, psq, 1]
        nc.vector.memset(P5jk[:], 1.0)
        nc.vector.tensor_copy(
            P5jk[:].rearrange("p (j k) -> p j k", k=5)[:, :, 0:3],
            pts96[:].rearrange("p (j c) -> p j c", c=3))
        nc.vector.tensor_copy(
            P5jk[:].rearrange("p (j k) -> p j k", k=5)[:, :, 3:4],
            psq[:].unsqueeze(2))


        # ---- P3 = W1b^T @ featT + (-2 W1a)^T @ pT ----
        p3_es = ExitStack()
        p3_psum = p3_es.enter_context(
            tc.tile_pool(name="p3_ps", bufs=2, space="PSUM"))
        for ci in range(NCHUNK):
            sl = slice(ci * CHUNK, (ci + 1) * CHUNK)
            ps = p3_psum.tile([C, CHUNK], F32, tag="p3ps")
            nc.tensor.matmul(ps[:], W1br[:], featT[:, sl], start=True,
                             stop=False)
            nc.tensor.matmul(ps[:], W1am2[:], pTr[:, sl], start=False,
                             stop=True)
            nc.scalar.copy(P3[:, sl], ps[:])
        p3_es.close()
        stage_es.close()

        # ---- pools for overlapped group work ----
        grp_es = ExitStack()
        small_ps = grp_es.enter_context(
            tc.tile_pool(name="small_ps", bufs=2, space="PSUM"))
        mlp_ps = grp_es.enter_context(
            tc.tile_pool(name="mlp_ps", bufs=3, space="PSUM"))
        h1_pool = grp_es.enter_context(tc.tile_pool(name="h1", bufs=4))
        mask_pool = grp_es.enter_context(tc.tile_pool(name="mask", bufs=2))
        msk_pool = grp_es.enter_context(tc.tile_pool(name="msk32", bufs=2))
        gp_pool = grp_es.enter_context(tc.tile_pool(name="gp", bufs=2))
        bx_pool = grp_es.enter_context(tc.tile_pool(name="bx", bufs=2))
        dram = es.enter_context(tc.tile_pool(name="dram", bufs=1,
                                             space="DRAM"))

        mask1 = {}
        mlp_tiles = {}

        # ================= FPS iteration =================
        def fps_iter(t):
            g, r = t // 8, t % 8
            if t > 0:
                s5prev = sgrp[:, 8 * (t - 1):8 * (t - 1) + 5]
                nc.vector.tensor_tensor(
                    dnm[:].rearrange("p (j k) -> p j k", k=5),
                    P5jk[:].rearrange("p (j k) -> p j k", k=5),
                    s5prev.unsqueeze(1).broadcast_to([128, NJ, 5]),
                    op=ALU.mult)
                nc.vector.tensor_reduce(
                    dn[:], dnm[:].rearrange("p (j k) -> p j k", k=5),
                    axis=AX.X, op=ALU.add)
                # t==1: min_d still holds the -index ramp used to pick point
                # 0 -> REPLACE with dn; afterwards accumulate the min
                if t == 1:
                    nc.vector.tensor_copy(min_d[:], dn[:])
                else:
                    nc.vector.tensor_tensor(min_d[:], min_d[:], dn[:],
                                            op=ALU.min)
            nc.vector.tensor_reduce(rowmax[:], min_d[:], axis=AX.X,
                                    op=ALU.max)
            nc.gpsimd.partition_all_reduce(gb[:], rowmax[:], channels=128,
                                           reduce_op=bass_isa.ReduceOp.max)
            # row-local argmax payload (overlaps the all-reduce)
            nc.vector.scalar_tensor_tensor(
                out=Rm[:].rearrange("p (k j) -> p k j", j=NJ),
                in0=min_d[:].unsqueeze(1).broadcast_to([128, 5, NJ]),
                scalar=rowmax[:],
                in1=X5kj[:].rearrange("p (k j) -> p k j", j=NJ),
                op0=ALU.is_ge, op1=ALU.mult)
            nc.vector.tensor_reduce(
                R5[:], Rm[:].rearrange("p (k j) -> p k j", j=NJ),
                axis=AX.X, op=ALU.add)
            nc.vector.scalar_tensor_tensor(
                out=P5s[:], in0=rowmax[:].broadcast_to([128, 5]),
                scalar=gb[:], in1=R5[:], op0=ALU.is_ge, op1=ALU.mult)
            nc.gpsimd.partition_all_reduce(
                sgrp[:, 8 * t:8 * t + 5], P5s[:], channels=128,
                reduce_op=bass_isa.ReduceOp.add)

        # ================= group work =================
        def emit_transform(g):
            i = g % 2
            # free-dim record block -> partitions, then per-core extract
            ps40 = small_ps.tile([64, 1], F32, tag="sm")
            nc.tensor.matmul(ps40[:], sgrp[0:1, 64 * g:64 * g + 64],
                             one1f[:], start=True, stop=True)
            nc.scalar.copy(sb40[i][:], ps40[:])
            ps5 = small_ps.tile([5, 1], F32, tag="sm")
            nc.tensor.matmul(ps5[:], sel540[:], sb40[i][:],
                             start=True, stop=True)
            # s5 = [-2c, 1, c^2]; mstat = s5 (mask stat); ctmo = [c; 1]
            nc.scalar.copy(mstat[i][:], ps5[:])
            nc.scalar.mul(ctmo[i][0:3, :], ps5[0:3, :], -0.5)
            psT15 = small_ps.tile([1, 5], F32, tag="sm")
            nc.tensor.transpose(psT15[:], mstat[i][:], ident[0:5, 0:5])
            nc.scalar.copy(s5row[i][:], psT15[:])
            psU = small_ps.tile([C, 1], F32, tag="sm")
            nc.tensor.matmul(psU[:], W1ab1[:], ctmo[i][:],
                             start=True, stop=True)
            nc.scalar.copy(U2bk[i][:], psU[:])
            if debug and g == 0:
                nc.sync.dma_start(d_dbg_U2b.ap(), U2bk[i][:])

        def emit_bcast(g):
            i = g % 2
            nc.gpsimd.partition_broadcast(s5own[i][:], s5row[i][:],
                                          channels=128)

        def emit_vmask(g):
            # exact fp32 d^2 in FPS layout -> bf16 step mask -> DRAM bounce
            # into the [1, N] row layout the NB inject consumes (n = 32p+j
            # so both DMAs are contiguous)
            i = g % 2
            nc.vector.tensor_tensor(
                d2m[:].rearrange("p (j k) -> p j k", k=5),
                P5jk[:].rearrange("p (j k) -> p j k", k=5),
                s5own[i][:].unsqueeze(1).broadcast_to([128, NJ, 5]),
                op=ALU.mult)
            nc.vector.tensor_reduce(
                d2g[:], d2m[:].rearrange("p (j k) -> p j k", k=5),
                axis=AX.X, op=ALU.add)
            msk = msk_pool.tile([128, NJ], BF16, tag="msk32")
            nc.vector.tensor_scalar(msk[:], d2g[:], THR, -BIGM,
                                    op0=ALU.is_ge, op1=ALU.mult)
            dmsk = dram.tile([128, NJ], BF16, name=f"dmsk{g % 2}")
            nc.sync.dma_start(dmsk[:], msk[:])
            mk = mask_pool.tile([1, N], BF16, tag="mask1")
            mask1[g] = mk
            nc.sync.dma_start(mk[:], dmsk[:].rearrange("p j -> (p j)"
                                                       ).unsqueeze(0))
            if debug and g == 0:
                nc.sync.dma_start(d_dbg_mask.ap(), mk[:])

        def emit_slot_w2(g, tiles):
            # W2 matmuls for psum tiles (each covers 2 chunks) of slot g
            i = g % 2
            for q in tiles:
                ps = mlp_ps.tile([C, 2 * CHUNK], F32, tag="mlp")
                mlp_tiles[(g, q)] = ps
                for half in range(2):
                    ci = 2 * q + half
                    sl = slice(ci * CHUNK, (ci + 1) * CHUNK)
                    qsl = slice(half * CHUNK, (half + 1) * CHUNK)
                    h1 = h1_pool.tile([C, CHUNK], F16, tag="h1")
                    nc.scalar.activation(h1[:], P3[:, sl], ACTF.Relu,
                                         bias=U2bk[i][:], scale=1.0)
                    nc.tensor.matmul(ps[:, qsl], W2h[:], h1[:],
                                     start=True, stop=False,
                                     skip_group_check=True)

        def emit_slot_nb(g, tiles):
            mk = mask1[g]
            for q in tiles:
                ps = mlp_tiles[(g, q)]
                for half in range(2):
                    ci = 2 * q + half
                    sl = slice(ci * CHUNK, (ci + 1) * CHUNK)
                    qsl = slice(half * CHUNK, (half + 1) * CHUNK)
                    nc.tensor.matmul(ps[:, qsl], ones128b[:],
                                     mk[0:1, sl], start=False, stop=True,
                                     skip_group_check=True)

        def emit_slot_reds(g):
            v = g % 4
            gparts = gp_pool.tile([C, 4], F32, tag="gparts")
            for q in range(4):
                if q == 3 and (g, 3) not in mlp_tiles:
                    emit_slot_w2(g, [3])
                    emit_slot_nb(g, [3])
                ps = mlp_tiles.pop((g, q))
                nc.vector.tensor_reduce(gparts[:, q:q + 1], ps[:],
                                        axis=AX.X, op=ALU.max)
            nc.vector.tensor_reduce(G4col[g // 4 % 2][:, v:v + 1], gparts[:],
                                    axis=AX.X, op=ALU.max)
            if debug and g == 3:
                nc.sync.dma_start(d_dbg_G.ap(), G4col[0][:])

        def emit_boxes(v):
            # box head for slots 4v..4v+3
            gc = G4col[v % 2]
            grelu = bx_pool.tile([C, 4], F32R, tag="bx")
            nc.scalar.activation(grelu[:], gc[:], ACTF.Relu, bias=b2c[:],
                                 scale=1.0)
            ps3 = small_ps.tile([C, 4], F32, tag="sm")
            nc.tensor.matmul(ps3[:], W3[:], grelu[:], start=True,
                             stop=True)
            g3 = bx_pool.tile([C, 4], F32R, tag="bx")
            nc.scalar.activation(g3[:], ps3[:], ACTF.Relu, bias=b3c[:],
                                 scale=1.0)
            ps4 = small_ps.tile([C, 4], F32, tag="sm")
            nc.tensor.matmul(ps4[:], W4[:], g3[:], start=True,
                             stop=True)
            g4 = bx_pool.tile([C, 4], F32R, tag="bx")
            nc.scalar.activation(g4[:], ps4[:], ACTF.Relu, bias=b4c[:],
                                 scale=1.0)
            psb = small_ps.tile([7, 4], F32, tag="sm")
            nc.tensor.matmul(psb[:], Wf[:], g4[:], start=True,
                             stop=False)
            nc.tensor.matmul(psb[:], bfr[:], ones1x4[:], start=False,
                             stop=True)
            nc.scalar.copy(boxT[:, 4 * v:4 * v + 4], psb[:])

        # ================= main schedule =================
        for t in range(M):
            fps_iter(t)
            g = t // 8
            if t % 8 == 7:
                emit_transform(g)
                emit_slot_w2(g, [0, 1, 2])
            if t % 8 == 0 and g >= 1:
                emit_bcast(g - 1)
            if t % 8 == 1 and g >= 1:
                emit_vmask(g - 1)
            if t % 8 == 2 and g >= 1:
                emit_slot_nb(g - 1, [0, 1, 2])
            if t % 8 == 3 and g >= 1:
                emit_slot_reds(g - 1)
                if (g - 1) % 4 == 3:
                    emit_boxes((g - 1) // 4)
        emit_bcast(NG - 1)
        emit_vmask(NG - 1)
        emit_slot_nb(NG - 1, [0, 1, 2])
        emit_slot_reds(NG - 1)
        emit_boxes(3)

        # ================= AllGather =================
        bounce_in = dram.tile([7, NG], F32, name="bnc_in")
        bounce_out = dram.tile([NCORES, 7 * NG], F32, name="bnc_out")
        nc.sync.dma_start(bounce_in[:], boxT[:])
        nc.gpsimd.collective_compute(
            "AllGather", mybir.AluOpType.bypass,
            replica_groups=[list(range(NCORES))],
            ins=[bounce_in[:].opt()],
            outs=[bounce_out[:].opt()],
        )
        # BTall[c, 8g+k] = bounce_out[k, 16c + g]
        nc.sync.dma_start(
            BTall[:].rearrange("c (g k) -> c g k", k=NCORES),
            bounce_out[:].rearrange("k (c g) -> c g k", g=NG),
        )
        if debug:
            nc.sync.dma_start(d_dbg_sgrp.ap(), sgrp[:])
            nc.sync.dma_start(d_dbg_boxT.ap(), boxT[:])
            nc.sync.dma_start(d_dbg_BT.ap(), BTall[:])
        grp_es.close()

        # ================= NMS =================
        nms_es = ExitStack()
        nms_psum = nms_es.enter_context(
            tc.tile_pool(name="nms_ps", bufs=1, space="PSUM"))
        S14 = cp.tile([14, 128], F32)
        BX = cp.tile([128, 14], F32)
        PR = cp.tile([128, 8], F32)
        TPs = cp.tile([8, 128], F32)
        ER = cp.tile([8, 8 * 128], F32)
        ER_i = cp.tile([8, 8 * 128], I32)
        P_s = cp.tile([128, 128], F32)
        keep = cp.tile([128, 2], F32)
        lo3 = cp.tile([128, 3], F32)
        hi3 = cp.tile([128, 3], F32)
        vol = cp.tile([128, 1], F32)
        outt = cp.tile([128, 6], F32)

        nc.gpsimd.iota(ER_i[:].rearrange("p (j c) -> p j c", c=128),
                       pattern=[[1, 8], [0, 128]], base=0,
                       channel_multiplier=-1)
        nc.vector.tensor_scalar(ER[:], ER_i[:], 0, None, op0=ALU.is_equal)

        nc.scalar.activation(S14[0:7, :], BTall[:], ACTF.Sigmoid)
        ps_bxall = nms_psum.tile([128, 14], F32, tag="bxall")
        nc.tensor.transpose(ps_bxall[:, 0:7], S14[0:7, :], ident[0:7, 0:7])
        nc.tensor.transpose(ps_bxall[:, 7:14], BTall[:], ident[0:7, 0:7])
        nc.vector.tensor_copy(BX[:], ps_bxall[:])
        # cols of BX: 0 score-sig, 1..3 center, 4..6 dims, 7 score-logit
        nc.vector.scalar_tensor_tensor(lo3[:], BX[:, 4:7], -0.5, BX[:, 1:4],
                                       op0=ALU.mult, op1=ALU.add)
        nc.vector.scalar_tensor_tensor(hi3[:], BX[:, 4:7], 0.5, BX[:, 1:4],
                                       op0=ALU.mult, op1=ALU.add)
        nc.vector.tensor_mul(vol[:], BX[:, 4:5], BX[:, 5:6])
        nc.vector.tensor_mul(vol[:], vol[:], BX[:, 6:7])
        nc.vector.tensor_copy(PR[:, 0:3], lo3[:])
        nc.vector.tensor_copy(PR[:, 3:6], hi3[:])
        nc.vector.tensor_copy(PR[:, 6:7], vol[:])
        nc.vector.tensor_copy(PR[:, 7:8], BX[:, 7:8])
        ps_tp = nms_psum.tile([8, 128], F32, tag="tp")
        nc.tensor.transpose(ps_tp[:], PR[:], ident[:])
        nc.vector.tensor_copy(TPs[:], ps_tp[:])
        psB = nms_psum.tile([128, 8 * 128], F32, tag="psB")
        for rr in range(8):
            nc.tensor.matmul(psB[:, rr * 128:(rr + 1) * 128],
                             ER[:, rr * 128:(rr + 1) * 128],
                             TPs[:], start=True, stop=True)

        def colB(rr):
            return psB[:, rr * 128:(rr + 1) * 128]

        wrk = nms_es.enter_context(tc.tile_pool(name="nms_wrk", bufs=1))
        inter = wrk.tile([128, 128], F32, tag="inter")
        tmpA = wrk.tile([128, 128], F32, tag="tmpA")
        tmpB = wrk.tile([128, 128], F32, tag="tmpB")
        for c in range(3):
            nc.vector.tensor_scalar(tmpA[:], colB(3 + c), hi3[:, c:c + 1],
                                    None, op0=ALU.min)
            nc.vector.tensor_scalar(tmpB[:], colB(c), lo3[:, c:c + 1], None,
                                    op0=ALU.max)
            nc.vector.scalar_tensor_tensor(tmpA[:], tmpB[:], -1.0, tmpA[:],
                                           op0=ALU.mult, op1=ALU.add)
            nc.vector.tensor_scalar_max(tmpA[:], tmpA[:], 0.0)
            if c == 0:
                nc.vector.tensor_copy(inter[:], tmpA[:])
            else:
                nc.vector.tensor_mul(inter[:], inter[:], tmpA[:])
        nc.vector.tensor_scalar(tmpB[:], colB(6), vol[:], 1e-8, op0=ALU.add,
                                op1=ALU.add)
        nc.vector.scalar_tensor_tensor(tmpB[:], inter[:], -1.0, tmpB[:],
                                       op0=ALU.mult, op1=ALU.add)
        nc.vector.scalar_tensor_tensor(tmpA[:], inter[:], 1.0 / NMS_THR,
                                       tmpB[:], op0=ALU.mult, op1=ALU.is_gt)
        nc.vector.tensor_scalar(tmpB[:], colB(7), BX[:, 7:8], None,
                                op0=ALU.is_lt)
        nc.vector.tensor_mul(P_s[:], tmpA[:], tmpB[:])
        if debug:
            nc.sync.dma_start(d_dbg_BX.ap(), BX[:])
            nc.sync.dma_start(d_dbg_Ps.ap(), P_s[:])
        nc.vector.memset(keep[:], 1.0)
        ps_k = nms_psum.tile([128, 2], F32, tag="kps")
        for it in range(NMS_ITERS):
            nc.tensor.matmul(ps_k[:], P_s[:], keep[:], start=True,
                             stop=True)
            if debug and it == 0:
                psk_dbg = cp.tile([128, 2], F32)
                nc.vector.tensor_copy(psk_dbg[:], ps_k[:])
                nc.sync.dma_start(d_dbg_psk.ap(), psk_dbg[:])
            nc.vector.tensor_scalar(keep[:], ps_k[:], 0.5, None,
                                    op0=ALU.is_lt)
            if debug and it == 0:
                nc.sync.dma_start(d_dbg_keep1.ap(), keep[:])
        if debug:
            nc.sync.dma_start(d_dbg_Ps2.ap(), P_s[:])
        if debug:
            nc.sync.dma_start(d_dbg_keep.ap(), keep[:, 0:1])
        nc.vector.tensor_scalar(outt[:], BX[:, 1:7], keep[:, 0:1], None,
                                op0=ALU.mult)
        nc.sync.dma_start(d_out.ap(), outt[:])

        nms_es.close()
        es.close()

    nc.compile()
    return nc


def _prep_inputs(vote_points, vote_features, W1, b1, W2, b2, W3, b3, W4, b4,
                 Wf, bf):
    f32 = np.float32
    pts = np.ascontiguousarray(vote_points, dtype=f32)
    feat = np.ascontiguousarray(vote_features, dtype=f32)
    base = {
        "pts96": pts.reshape(128, 96).copy(),
        "pT": pts.T.copy(),
        "featT": feat.T.copy(),
        "W1a": np.ascontiguousarray(W1[:3], f32),
        "W1b": np.ascontiguousarray(W1[3:], f32),
        "W2": np.ascontiguousarray(W2, f32),
        "W3": np.ascontiguousarray(W3, f32),
        "W4": np.ascontiguousarray(W4, f32),
        "Wf": np.ascontiguousarray(Wf, f32),
        "b1r": np.ascontiguousarray(b1, f32).reshape(1, C),
        "b2c": np.ascontiguousarray(b2, f32).reshape(C, 1),
        "b3c": np.ascontiguousarray(b3, f32).reshape(C, 1),
        "b4c": np.ascontiguousarray(b4, f32).reshape(C, 1),
        "bfr": np.ascontiguousarray(bf, f32).reshape(1, 7),
    }
    in_maps = []
    for k in range(NCORES):
        m = dict(base)
        sel = np.zeros((64, 5), f32)
        for c in range(5):
            sel[8 * k + c, c] = 1.0
        m["sel540"] = sel
        in_maps.append(m)
    return in_maps


def kernel(**inputs):
    from concourse.bass_utils import run_bass_kernel_spmd

    if "nc" not in _cache:
        _cache["nc"] = _build(debug=False)
    nc = _cache["nc"]
    in_maps = _prep_inputs(**inputs)
    res = run_bass_kernel_spmd(nc, in_maps, core_ids=list(range(NCORES)))
    out = np.asarray(res.results[0]["out"], dtype=np.float32)
    return out


# revision 31
# speedup vs baseline: 1.0398x; 1.0398x over previous
"""Trainium2 Bass kernel for nn_DetectionHead (VoteNet-style detection head).

Self-contained 8-core SPMD kernel. Strategy:
  - FPS (128 serial argmax iterations) is the critical path. Each iteration
    is a fused Vector/GpSimd chain: tensor_tensor_reduce fuses the min_d
    update with the row-max; the per-row argmax payload (masked sums of
    [-2p, 1, |p|^2]) is computed while GpSimd runs the max all-reduce; the
    add all-reduce then both broadcasts the winner payload AND records it
    (each iteration writes its own 5-column slice of `sgrp`).
  - Clusters are processed in 16 fine groups (1 cluster per core per group,
    cluster m = 8*g + core) overlapped under FPS: per-group d^2 mask via a
    single stacked [5,x]@[5,512] matmul per chunk, fp16 W2 layer, -BIG mask
    injection, chunked max-pool, batched box head.
  - One AllGather of the per-core [7,16] box logits, then replicated NMS.

kernel(**inputs) takes the full unsharded inputs, returns the full [128,6]
output.
"""

import numpy as np

NCORES = 8
N = 4096          # points
C = 128           # feature channels
M = 128           # clusters
NJ = 32           # points per partition (N = 128 * NJ)
NG = 16           # cluster groups (1 cluster per core per group)
RADIUS = 0.5
THR = RADIUS * RADIUS
NMS_THR = 0.25
BIGM = 8388608.0  # 2^23, exact in bf16/f32r
NMS_ITERS = 6
CHUNK = 512
NCHUNK = N // CHUNK       # 8

_cache = {}


def _build(debug=False):
    import concourse.bacc as bacc
    import concourse.tile as tile
    import concourse.mybir as mybir
    import concourse.bass_isa as bass_isa

    F32 = mybir.dt.float32
    F32R = mybir.dt.float32r
    F16 = mybir.dt.float16
    BF16 = mybir.dt.bfloat16
    I32 = mybir.dt.int32
    ALU = mybir.AluOpType
    ACTF = mybir.ActivationFunctionType
    AX = mybir.AxisListType
    FLT_MAX = 3.4028234e38

    nc = bacc.Bacc("TRN2", target_bir_lowering=False, debug=False,
                   num_devices=NCORES)

    # ---- DRAM I/O ----
    d_pts96 = nc.dram_tensor("pts96", [128, 96], F32, kind="ExternalInput")
    d_pT = nc.dram_tensor("pT", [3, N], F32, kind="ExternalInput")
    d_featT = nc.dram_tensor("featT", [C, N], F32, kind="ExternalInput")
    d_W1a = nc.dram_tensor("W1a", [3, C], F32, kind="ExternalInput")
    d_W1b = nc.dram_tensor("W1b", [C, C], F32, kind="ExternalInput")
    d_W2 = nc.dram_tensor("W2", [C, C], F32, kind="ExternalInput")
    d_W3 = nc.dram_tensor("W3", [C, C], F32, kind="ExternalInput")
    d_W4 = nc.dram_tensor("W4", [C, C], F32, kind="ExternalInput")
    d_Wf = nc.dram_tensor("Wf", [C, 7], F32, kind="ExternalInput")
    d_b1r = nc.dram_tensor("b1r", [1, C], F32, kind="ExternalInput")
    d_b2c = nc.dram_tensor("b2c", [C, 1], F32, kind="ExternalInput")
    d_b3c = nc.dram_tensor("b3c", [C, 1], F32, kind="ExternalInput")
    d_b4c = nc.dram_tensor("b4c", [C, 1], F32, kind="ExternalInput")
    d_bfr = nc.dram_tensor("bfr", [1, 7], F32, kind="ExternalInput")
    d_sel540 = nc.dram_tensor("sel540", [64, 5], F32, kind="ExternalInput")

    d_out = nc.dram_tensor("out", [M, 6], F32, kind="ExternalOutput")
    if debug:
        d_dbg_sgrp = nc.dram_tensor("dbg_sgrp", [128, 8 * M], F32,
                                    kind="ExternalOutput")
        d_dbg_boxT = nc.dram_tensor("dbg_boxT", [7, NG], F32,
                                    kind="ExternalOutput")
        d_dbg_BT = nc.dram_tensor("dbg_BT", [7, 128], F32,
                                  kind="ExternalOutput")
        d_dbg_keep = nc.dram_tensor("dbg_keep", [128, 1], F32,
                                    kind="ExternalOutput")
        d_dbg_U2b = nc.dram_tensor("dbg_U2b", [128, 1], F32,
                                   kind="ExternalOutput")
        d_dbg_mask = nc.dram_tensor("dbg_mask", [1, N], BF16,
                                    kind="ExternalOutput")
        d_dbg_BX = nc.dram_tensor("dbg_BX", [128, 14], F32,
                                  kind="ExternalOutput")
        d_dbg_Ps = nc.dram_tensor("dbg_Ps", [128, 128], F32,
                                  kind="ExternalOutput")
        d_dbg_G = nc.dram_tensor("dbg_G", [128, 4], F32,
                                 kind="ExternalOutput")
        d_dbg_psk = nc.dram_tensor("dbg_psk", [128, 2], F32,
                                   kind="ExternalOutput")
        d_dbg_keep1 = nc.dram_tensor("dbg_keep1", [128, 2], F32,
                                     kind="ExternalOutput")
        d_dbg_Ps2 = nc.dram_tensor("dbg_Ps2", [128, 128], F32,
                                   kind="ExternalOutput")

    from contextlib import ExitStack
    es = ExitStack()
    with tile.TileContext(nc) as tc:
        cp = es.enter_context(tc.tile_pool(name="const", bufs=1))
        stage_es = ExitStack()
        stage = stage_es.enter_context(tc.tile_pool(name="stage", bufs=1))

        # ---- persistent tiles ----
        pts96 = cp.tile([128, 96], F32)
        pT = cp.tile([3, N], F32)
        featT = stage.tile([C, N], F32R, name="featTr")
        X5kj = cp.tile([128, 5 * NJ], F32)    # k-major: [-2p | 1 | psq]
        P5jk = cp.tile([128, NJ * 5], F32)    # j-major: [p | psq | 1]
        psq = cp.tile([128, NJ], F32)
        min_d = cp.tile([128, NJ], F32)
        junk32 = cp.tile([128, NJ], F32)
        dnm = cp.tile([128, NJ * 5], F32)
        dn = cp.tile([128, NJ], F32)
        rowmax = cp.tile([128, 1], F32)
        gb = cp.tile([128, 1], F32)
        Rm = cp.tile([128, 5 * NJ], F32)
        R5 = cp.tile([128, 5], F32)
        P5s = cp.tile([128, 5], F32)
        sgrp = cp.tile([128, 8 * M], F32)     # 8-col (32B) aligned records
        sel540 = cp.tile([64, 5], F32)        # per-core record extractor
        sb40 = [cp.tile([64, 1], F32, name=f"sb40_{i}") for i in range(2)]
        one1f = cp.tile([1, 1], F32)
        W1ab1 = cp.tile([4, C], F32)          # [W1a; b1]
        W1am2 = cp.tile([3, C], F32R)         # -2 * W1a
        W2h = cp.tile([C, C], F16)
        W3 = cp.tile([C, C], F32R)
        W4 = cp.tile([C, C], F32R)
        Wf = cp.tile([C, 7], F32R)
        b2c = cp.tile([C, 1], F32)
        b3c = cp.tile([C, 1], F32)
        b4c = cp.tile([C, 1], F32)
        bfr = cp.tile([1, 7], F32R)
        ones128b = cp.tile([1, 128], BF16)    # NB-inject stationary
        ones1x4 = cp.tile([1, 4], F32R)
        s5row = [cp.tile([1, 5], F32, name=f"s5row{i}") for i in range(2)]
        s5own = [cp.tile([128, 5], F32, name=f"s5own{i}") for i in range(2)]
        d2m = cp.tile([128, NJ * 5], F32)
        d2g = cp.tile([128, NJ], F32)
        ident = cp.tile([128, 128], F32)
        ident_i = cp.tile([128, 128], I32)
        iotaN_i = cp.tile([128, NJ], I32)
        P3 = cp.tile([C, N], F32)
        ctmo = [cp.tile([4, 1], F32, name=f"ctmo{i}") for i in range(2)]
        mstat = [cp.tile([5, 1], F32, name=f"mstat{i}") for i in range(2)]
        U2bk = [cp.tile([C, 1], F32, name=f"u2bk{i}") for i in range(2)]
        G4col = [cp.tile([128, 4], F32, name=f"g4col{i}") for i in range(2)]
        boxT = cp.tile([7, NG], F32)
        BTall = cp.tile([7, 128], F32)

        # stage tiles (freed after setup)
        tmp96 = stage.tile([128, 96], F32)

        def r_(ap):
            return ap.bitcast(F32R)

        # ---- input DMA ----
        nc.sync.dma_start(pts96[:], d_pts96.ap())
        nc.sync.dma_start(pT[:], d_pT.ap())
        featT_st = stage.tile([C, N], F32, name="featT_st")
        pTr = stage.tile([3, N], F32R, name="pTr")
        W1b_st = stage.tile([C, C], F32, name="w1b_st")
        W1br = stage.tile([C, C], F32R, name="w1br")
        W3_st = stage.tile([C, C], F32, name="w3_st")
        W4_st = stage.tile([C, C], F32, name="w4_st")
        Wf_st = stage.tile([C, 7], F32, name="wf_st")
        bfr_st = stage.tile([1, 7], F32, name="bfr_st")
        nc.sync.dma_start(featT_st[:], d_featT.ap())
        nc.sync.dma_start(W1ab1[0:3, :], d_W1a.ap())
        nc.sync.dma_start(W1ab1[3:4, :], d_b1r.ap())
        nc.sync.dma_start(W1b_st[:], d_W1b.ap())
        W2h_stage = stage.tile([C, C], F32, name="w2stage")
        nc.sync.dma_start(W2h_stage[:], d_W2.ap())
        nc.sync.dma_start(W3_st[:], d_W3.ap())
        nc.sync.dma_start(W4_st[:], d_W4.ap())
        nc.sync.dma_start(Wf_st[:], d_Wf.ap())
        nc.sync.dma_start(b2c[:], d_b2c.ap())
        nc.sync.dma_start(b3c[:], d_b3c.ap())
        nc.sync.dma_start(b4c[:], d_b4c.ap())
        nc.sync.dma_start(bfr_st[:], d_bfr.ap())
        nc.sync.dma_start(sel540[:], d_sel540.ap())

        # ---- constants ----
        nc.gpsimd.iota(ident_i[:], pattern=[[1, 128]], base=0,
                       channel_multiplier=-1)
        nc.vector.tensor_scalar(ident[:], ident_i[:], 0, None,
                                op0=ALU.is_equal)
        nc.gpsimd.iota(iotaN_i[:], pattern=[[1, NJ]], base=0,
                       channel_multiplier=NJ)
        nc.vector.tensor_scalar(min_d[:], iotaN_i[:], -1.0, None,
                                op0=ALU.mult)
        o4f = stage.tile([1, 4], F32, name="o4f")
        ob = stage.tile([1, 128], F32, name="ob")
        nc.vector.memset(o4f[:], 1.0)
        nc.vector.tensor_copy(ones1x4[:], o4f[:])
        nc.vector.memset(ob[:], 1.0)
        nc.vector.tensor_copy(ones128b[:], ob[:])
        nc.vector.memset(one1f[:], 1.0)
        nc.vector.tensor_copy(W2h[:], W2h_stage[:])
        nc.vector.tensor_copy(featT[:], featT_st[:])
        nc.scalar.copy(pTr[:], pT[:])
        nc.vector.tensor_copy(W1br[:], W1b_st[:])
        nc.vector.tensor_copy(W3[:], W3_st[:])
        nc.vector.tensor_copy(W4[:], W4_st[:])
        nc.vector.tensor_copy(Wf[:], Wf_st[:])
        nc.vector.tensor_copy(bfr[:], bfr_st[:])
        nc.scalar.mul(W1am2[:], W1ab1[0:3, :], -2.0)
        for i in range(2):
            nc.vector.memset(ctmo[i][:], 1.0)  # row 3 stays 1.0

        # psq[p, j] = |p_n|^2 for n = 32p + j
        nc.vector.tensor_mul(tmp96[:], pts96[:], pts96[:])
        nc.vector.tensor_reduce(
            psq[:], tmp96[:].rearrange("p (j c) -> p j c", c=3),
            axis=AX.X, op=ALU.add)

        # X5kj: k-major rows [-2px, -2py, -2pz, 1, psq]
        nc.scalar.mul(
            X5kj[:].rearrange("p (k j) -> p k j", j=NJ)[:, 0:3, :],
            pts96[:].rearrange("p (j c) -> p c j", c=3), -2.0)
        nc.vector.memset(X5kj[:, 3 * NJ:4 * NJ], 1.0)
        nc.vector.tensor_copy(X5kj[:, 4 * NJ:5 * NJ], psq[:])

        # P5jk: j-major rows [px, py, pz, psq, 1]
        nc.vector.memset(P5jk[:], 1.0)
        nc.vector.tensor_copy(
            P5jk[:].rearrange("p (j k) -> p j k", k=5)[:, :, 0:3],
            pts96[:].rearrange("p (j c) -> p j c", c=3))
        nc.vector.tensor_copy(
            P5jk[:].rearrange("p (j k) -> p j k", k=5)[:, :, 3:4],
            psq[:].unsqueeze(2))


        # ---- P3 = W1b^T @ featT + (-2 W1a)^T @ pT ----
        p3_es = ExitStack()
        p3_psum = p3_es.enter_context(
            tc.tile_pool(name="p3_ps", bufs=2, space="PSUM"))
        for ci in range(NCHUNK):
            sl = slice(ci * CHUNK, (ci + 1) * CHUNK)
            ps = p3_psum.tile([C, CHUNK], F32, tag="p3ps")
            nc.tensor.matmul(ps[:], W1br[:], featT[:, sl], start=True,
                             stop=False)
            nc.tensor.matmul(ps[:], W1am2[:], pTr[:, sl], start=False,
                             stop=True)
            nc.scalar.copy(P3[:, sl], ps[:])
        p3_es.close()
        stage_es.close()

        # ---- pools for overlapped group work ----
        grp_es = ExitStack()
        small_ps = grp_es.enter_context(
            tc.tile_pool(name="small_ps", bufs=2, space="PSUM"))
        mlp_ps = grp_es.enter_context(
            tc.tile_pool(name="mlp_ps", bufs=3, space="PSUM"))
        h1_pool = grp_es.enter_context(tc.tile_pool(name="h1", bufs=4))
        mask_pool = grp_es.enter_context(tc.tile_pool(name="mask", bufs=2))
        msk_pool = grp_es.enter_context(tc.tile_pool(name="msk32", bufs=2))
        gp_pool = grp_es.enter_context(tc.tile_pool(name="gp", bufs=2))
        hq_pool = grp_es.enter_context(tc.tile_pool(name="hq", bufs=2))
        bx_pool = grp_es.enter_context(tc.tile_pool(name="bx", bufs=2))
        dram = es.enter_context(tc.tile_pool(name="dram", bufs=1,
                                             space="DRAM"))

        mask1 = {}
        mlp_tiles = {}

        # ================= FPS iteration =================
        def fps_iter(t):
            g, r = t // 8, t % 8
            if t > 0:
                s5prev = sgrp[:, 8 * (t - 1):8 * (t - 1) + 5]
                nc.vector.tensor_tensor(
                    dnm[:].rearrange("p (j k) -> p j k", k=5),
                    P5jk[:].rearrange("p (j k) -> p j k", k=5),
                    s5prev.unsqueeze(1).broadcast_to([128, NJ, 5]),
                    op=ALU.mult)
                nc.vector.tensor_reduce(
                    dn[:], dnm[:].rearrange("p (j k) -> p j k", k=5),
                    axis=AX.X, op=ALU.add)
                # t==1: min_d still holds the -index ramp used to pick point
                # 0 -> REPLACE with dn; afterwards accumulate the min
                if t == 1:
                    nc.vector.tensor_copy(min_d[:], dn[:])
                else:
                    nc.vector.tensor_tensor(min_d[:], min_d[:], dn[:],
                                            op=ALU.min)
            nc.vector.tensor_reduce(rowmax[:], min_d[:], axis=AX.X,
                                    op=ALU.max)
            nc.gpsimd.partition_all_reduce(gb[:], rowmax[:], channels=128,
                                           reduce_op=bass_isa.ReduceOp.max)
            # row-local argmax payload (overlaps the all-reduce)
            nc.vector.scalar_tensor_tensor(
                out=Rm[:].rearrange("p (k j) -> p k j", j=NJ),
                in0=min_d[:].unsqueeze(1).broadcast_to([128, 5, NJ]),
                scalar=rowmax[:],
                in1=X5kj[:].rearrange("p (k j) -> p k j", j=NJ),
                op0=ALU.is_ge, op1=ALU.mult)
            nc.vector.tensor_reduce(
                R5[:], Rm[:].rearrange("p (k j) -> p k j", j=NJ),
                axis=AX.X, op=ALU.add)
            nc.vector.scalar_tensor_tensor(
                out=P5s[:], in0=rowmax[:].broadcast_to([128, 5]),
                scalar=gb[:], in1=R5[:], op0=ALU.is_ge, op1=ALU.mult)
            nc.gpsimd.partition_all_reduce(
                sgrp[:, 8 * t:8 * t + 5], P5s[:], channels=128,
                reduce_op=bass_isa.ReduceOp.add)

        # ================= group work =================
        def emit_transform(g):
            i = g % 2
            # free-dim record block -> partitions, then per-core extract
            ps40 = small_ps.tile([64, 1], F32, tag="sm")
            nc.tensor.matmul(ps40[:], sgrp[0:1, 64 * g:64 * g + 64],
                             one1f[:], start=True, stop=True)
            nc.scalar.copy(sb40[i][:], ps40[:])
            ps5 = small_ps.tile([5, 1], F32, tag="sm")
            nc.tensor.matmul(ps5[:], sel540[:], sb40[i][:],
                             start=True, stop=True)
            # s5 = [-2c, 1, c^2]; mstat = s5 (mask stat); ctmo = [c; 1]
            nc.scalar.copy(mstat[i][:], ps5[:])
            nc.scalar.mul(ctmo[i][0:3, :], ps5[0:3, :], -0.5)
            psT15 = small_ps.tile([1, 5], F32, tag="sm")
            nc.tensor.transpose(psT15[:], mstat[i][:], ident[0:5, 0:5])
            nc.scalar.copy(s5row[i][:], psT15[:])
            psU = small_ps.tile([C, 1], F32, tag="sm")
            nc.tensor.matmul(psU[:], W1ab1[:], ctmo[i][:],
                             start=True, stop=True)
            nc.scalar.copy(U2bk[i][:], psU[:])
            if debug and g == 0:
                nc.sync.dma_start(d_dbg_U2b.ap(), U2bk[i][:])

        def emit_bcast(g):
            i = g % 2
            nc.gpsimd.partition_broadcast(s5own[i][:], s5row[i][:],
                                          channels=128)

        def emit_vmask(g):
            # exact fp32 d^2 in FPS layout -> bf16 step mask -> DRAM bounce
            # into the [1, N] row layout the NB inject consumes (n = 32p+j
            # so both DMAs are contiguous)
            i = g % 2
            nc.vector.tensor_tensor(
                d2m[:].rearrange("p (j k) -> p j k", k=5),
                P5jk[:].rearrange("p (j k) -> p j k", k=5),
                s5own[i][:].unsqueeze(1).broadcast_to([128, NJ, 5]),
                op=ALU.mult)
            nc.vector.tensor_reduce(
                d2g[:], d2m[:].rearrange("p (j k) -> p j k", k=5),
                axis=AX.X, op=ALU.add)
            msk = msk_pool.tile([128, NJ], BF16, tag="msk32")
            nc.vector.tensor_scalar(msk[:], d2g[:], THR, -BIGM,
                                    op0=ALU.is_ge, op1=ALU.mult)
            dmsk = dram.tile([128, NJ], BF16, name=f"dmsk{g % 2}")
            nc.sync.dma_start(dmsk[:], msk[:])
            mk = mask_pool.tile([1, N], BF16, tag="mask1")
            mask1[g] = mk
            nc.sync.dma_start(mk[:], dmsk[:].rearrange("p j -> (p j)"
                                                       ).unsqueeze(0))
            if debug and g == 0:
                nc.sync.dma_start(d_dbg_mask.ap(), mk[:])

        def emit_slot_w2(g, tiles):
            # W2 matmuls for psum tiles (each covers 2 chunks) of slot g
            i = g % 2
            for q in tiles:
                ps = mlp_ps.tile([C, 2 * CHUNK], F32, tag="mlp")
                mlp_tiles[(g, q)] = ps
                for half in range(2):
                    ci = 2 * q + half
                    sl = slice(ci * CHUNK, (ci + 1) * CHUNK)
                    qsl = slice(half * CHUNK, (half + 1) * CHUNK)
                    h1 = h1_pool.tile([C, CHUNK], F16, tag="h1")
                    nc.scalar.activation(h1[:], P3[:, sl], ACTF.Relu,
                                         bias=U2bk[i][:], scale=1.0)
                    nc.tensor.matmul(ps[:, qsl], W2h[:], h1[:],
                                     start=True, stop=False,
                                     skip_group_check=True)

        def emit_slot_nb(g, tiles):
            mk = mask1[g]
            for q in tiles:
                ps = mlp_tiles[(g, q)]
                for half in range(2):
                    ci = 2 * q + half
                    sl = slice(ci * CHUNK, (ci + 1) * CHUNK)
                    qsl = slice(half * CHUNK, (half + 1) * CHUNK)
                    nc.tensor.matmul(ps[:, qsl], ones128b[:],
                                     mk[0:1, sl], start=False, stop=True,
                                     skip_group_check=True)

        gparts2 = [cp.tile([C, 4], F32, name=f"gparts{i}")
                   for i in range(2)]

        def emit_slot_red_q(g, q):
            # stage psum tile -> fp16 on Scalar, max-reduce fp16 on Vector
            if q == 3 and (g, 3) not in mlp_tiles:
                emit_slot_w2(g, [3])
                emit_slot_nb(g, [3])
            ps = mlp_tiles.pop((g, q))
            hcp = hq_pool.tile([C, 2 * CHUNK], F16, tag="hq")
            nc.scalar.copy(hcp[:], ps[:])
            nc.vector.tensor_reduce(gparts2[g % 2][:, q:q + 1], hcp[:],
                                    axis=AX.X, op=ALU.max)

        def emit_combine(g):
            v = g % 4
            nc.vector.tensor_reduce(G4col[g // 4 % 2][:, v:v + 1],
                                    gparts2[g % 2][:], axis=AX.X, op=ALU.max)
            if debug and g == 3:
                nc.sync.dma_start(d_dbg_G.ap(), G4col[0][:])

        def emit_boxes(v):
            # box head for slots 4v..4v+3
            gc = G4col[v % 2]
            grelu = bx_pool.tile([C, 4], F32R, tag="bx")
            nc.scalar.activation(grelu[:], gc[:], ACTF.Relu, bias=b2c[:],
                                 scale=1.0)
            ps3 = small_ps.tile([C, 4], F32, tag="sm")
            nc.tensor.matmul(ps3[:], W3[:], grelu[:], start=True,
                             stop=True)
            g3 = bx_pool.tile([C, 4], F32R, tag="bx")
            nc.scalar.activation(g3[:], ps3[:], ACTF.Relu, bias=b3c[:],
                                 scale=1.0)
            ps4 = small_ps.tile([C, 4], F32, tag="sm")
            nc.tensor.matmul(ps4[:], W4[:], g3[:], start=True,
                             stop=True)
            g4 = bx_pool.tile([C, 4], F32R, tag="bx")
            nc.scalar.activation(g4[:], ps4[:], ACTF.Relu, bias=b4c[:],
                                 scale=1.0)
            psb = small_ps.tile([7, 4], F32, tag="sm")
            nc.tensor.matmul(psb[:], Wf[:], g4[:], start=True,
                             stop=False)
            nc.tensor.matmul(psb[:], bfr[:], ones1x4[:], start=False,
                             stop=True)
            nc.scalar.copy(boxT[:, 4 * v:4 * v + 4], psb[:])

        # ================= main schedule =================
        for t in range(M):
            fps_iter(t)
            g = t // 8
            if t % 8 == 7:
                if g >= 1:
                    emit_combine(g - 1)
                    if (g - 1) % 4 == 3:
                        emit_boxes((g - 1) // 4)
                emit_transform(g)
                emit_slot_w2(g, [0, 1, 2])
            if t % 8 == 0 and g >= 1:
                emit_bcast(g - 1)
            if t % 8 == 1 and g >= 1:
                emit_vmask(g - 1)
            if t % 8 == 2 and g >= 1:
                emit_slot_nb(g - 1, [0, 1, 2])
            if 3 <= t % 8 <= 6 and g >= 1:
                emit_slot_red_q(g - 1, t % 8 - 3)
        emit_bcast(NG - 1)
        emit_vmask(NG - 1)
        emit_slot_nb(NG - 1, [0, 1, 2])
        for q in range(4):
            emit_slot_red_q(NG - 1, q)
        emit_combine(NG - 1)
        emit_boxes(3)

        # ================= AllGather =================
        bounce_in = dram.tile([7, NG], F32, name="bnc_in")
        bounce_out = dram.tile([NCORES, 7 * NG], F32, name="bnc_out")
        nc.sync.dma_start(bounce_in[:], boxT[:])
        nc.gpsimd.collective_compute(
            "AllGather", mybir.AluOpType.bypass,
            replica_groups=[list(range(NCORES))],
            ins=[bounce_in[:].opt()],
            outs=[bounce_out[:].opt()],
        )
        # BTall[c, 8g+k] = bounce_out[k, 16c + g]
        nc.sync.dma_start(
            BTall[:].rearrange("c (g k) -> c g k", k=NCORES),
            bounce_out[:].rearrange("k (c g) -> c g k", g=NG),
        )
        if debug:
            nc.sync.dma_start(d_dbg_sgrp.ap(), sgrp[:])
            nc.sync.dma_start(d_dbg_boxT.ap(), boxT[:])
            nc.sync.dma_start(d_dbg_BT.ap(), BTall[:])
        grp_es.close()

        # ================= NMS =================
        nms_es = ExitStack()
        nms_psum = nms_es.enter_context(
            tc.tile_pool(name="nms_ps", bufs=1, space="PSUM"))
        S14 = cp.tile([14, 128], F32)
        BX = cp.tile([128, 14], F32)
        PR = cp.tile([128, 8], F32)
        TPs = cp.tile([8, 128], F32)
        ER = cp.tile([8, 8 * 128], F32)
        ER_i = cp.tile([8, 8 * 128], I32)
        P_s = cp.tile([128, 128], F32)
        keep = cp.tile([128, 2], F32)
        lo3 = cp.tile([128, 3], F32)
        hi3 = cp.tile([128, 3], F32)
        vol = cp.tile([128, 1], F32)
        outt = cp.tile([128, 6], F32)

        nc.gpsimd.iota(ER_i[:].rearrange("p (j c) -> p j c", c=128),
                       pattern=[[1, 8], [0, 128]], base=0,
                       channel_multiplier=-1)
        nc.vector.tensor_scalar(ER[:], ER_i[:], 0, None, op0=ALU.is_equal)

        nc.scalar.activation(S14[0:7, :], BTall[:], ACTF.Sigmoid)
        ps_bxall = nms_psum.tile([128, 14], F32, tag="bxall")
        nc.tensor.transpose(ps_bxall[:, 0:7], S14[0:7, :], ident[0:7, 0:7])
        nc.tensor.transpose(ps_bxall[:, 7:14], BTall[:], ident[0:7, 0:7])
        nc.vector.tensor_copy(BX[:], ps_bxall[:])
        # cols of BX: 0 score-sig, 1..3 center, 4..6 dims, 7 score-logit
        nc.vector.scalar_tensor_tensor(lo3[:], BX[:, 4:7], -0.5, BX[:, 1:4],
                                       op0=ALU.mult, op1=ALU.add)
        nc.vector.scalar_tensor_tensor(hi3[:], BX[:, 4:7], 0.5, BX[:, 1:4],
                                       op0=ALU.mult, op1=ALU.add)
        nc.vector.tensor_mul(vol[:], BX[:, 4:5], BX[:, 5:6])
        nc.vector.tensor_mul(vol[:], vol[:], BX[:, 6:7])
        nc.vector.tensor_copy(PR[:, 0:3], lo3[:])
        nc.vector.tensor_copy(PR[:, 3:6], hi3[:])
        nc.vector.tensor_copy(PR[:, 6:7], vol[:])
        nc.vector.tensor_copy(PR[:, 7:8], BX[:, 7:8])
        ps_tp = nms_psum.tile([8, 128], F32, tag="tp")
        nc.tensor.transpose(ps_tp[:], PR[:], ident[:])
        nc.vector.tensor_copy(TPs[:], ps_tp[:])
        psB = nms_psum.tile([128, 8 * 128], F32, tag="psB")
        for rr in range(8):
            nc.tensor.matmul(psB[:, rr * 128:(rr + 1) * 128],
                             ER[:, rr * 128:(rr + 1) * 128],
                             TPs[:], start=True, stop=True)

        def colB(rr):
            return psB[:, rr * 128:(rr + 1) * 128]

        wrk = nms_es.enter_context(tc.tile_pool(name="nms_wrk", bufs=1))
        inter = wrk.tile([128, 128], F32, tag="inter")
        tmpA = wrk.tile([128, 128], F32, tag="tmpA")
        tmpB = wrk.tile([128, 128], F32, tag="tmpB")
        for c in range(3):
            nc.vector.tensor_scalar(tmpA[:], colB(3 + c), hi3[:, c:c + 1],
                                    None, op0=ALU.min)
            nc.vector.tensor_scalar(tmpB[:], colB(c), lo3[:, c:c + 1], None,
                                    op0=ALU.max)
            nc.vector.scalar_tensor_tensor(tmpA[:], tmpB[:], -1.0, tmpA[:],
                                           op0=ALU.mult, op1=ALU.add)
            nc.vector.tensor_scalar_max(tmpA[:], tmpA[:], 0.0)
            if c == 0:
                nc.vector.tensor_copy(inter[:], tmpA[:])
            else:
                nc.vector.tensor_mul(inter[:], inter[:], tmpA[:])
        nc.vector.tensor_scalar(tmpB[:], colB(6), vol[:], 1e-8, op0=ALU.add,
                                op1=ALU.add)
        nc.vector.scalar_tensor_tensor(tmpB[:], inter[:], -1.0, tmpB[:],
                                       op0=ALU.mult, op1=ALU.add)
        nc.vector.scalar_tensor_tensor(tmpA[:], inter[:], 1.0 / NMS_THR,
                                       tmpB[:], op0=ALU.mult, op1=ALU.is_gt)
        nc.vector.tensor_scalar(tmpB[:], colB(7), BX[:, 7:8], None,
                                op0=ALU.is_lt)
        nc.vector.tensor_mul(P_s[:], tmpA[:], tmpB[:])
        if debug:
            nc.sync.dma_start(d_dbg_BX.ap(), BX[:])
            nc.sync.dma_start(d_dbg_Ps.ap(), P_s[:])
        nc.vector.memset(keep[:], 1.0)
        ps_k = nms_psum.tile([128, 2], F32, tag="kps")
        for it in range(NMS_ITERS):
            nc.tensor.matmul(ps_k[:], P_s[:], keep[:], start=True,
                             stop=True)
            if debug and it == 0:
                psk_dbg = cp.tile([128, 2], F32)
                nc.vector.tensor_copy(psk_dbg[:], ps_k[:])
                nc.sync.dma_start(d_dbg_psk.ap(), psk_dbg[:])
            nc.vector.tensor_scalar(keep[:], ps_k[:], 0.5, None,
                                    op0=ALU.is_lt)
            if debug and it == 0:
                nc.sync.dma_start(d_dbg_keep1.ap(), keep[:])
        if debug:
            nc.sync.dma_start(d_dbg_Ps2.ap(), P_s[:])
        if debug:
            nc.sync.dma_start(d_dbg_keep.ap(), keep[:, 0:1])
        nc.vector.tensor_scalar(outt[:], BX[:, 1:7], keep[:, 0:1], None,
                                op0=ALU.mult)
        nc.sync.dma_start(d_out.ap(), outt[:])

        nms_es.close()
        es.close()

    nc.compile()
    return nc


def _prep_inputs(vote_points, vote_features, W1, b1, W2, b2, W3, b3, W4, b4,
                 Wf, bf):
    f32 = np.float32
    pts = np.ascontiguousarray(vote_points, dtype=f32)
    feat = np.ascontiguousarray(vote_features, dtype=f32)
    base = {
        "pts96": pts.reshape(128, 96).copy(),
        "pT": pts.T.copy(),
        "featT": feat.T.copy(),
        "W1a": np.ascontiguousarray(W1[:3], f32),
        "W1b": np.ascontiguousarray(W1[3:], f32),
        "W2": np.ascontiguousarray(W2, f32),
        "W3": np.ascontiguousarray(W3, f32),
        "W4": np.ascontiguousarray(W4, f32),
        "Wf": np.ascontiguousarray(Wf, f32),
        "b1r": np.ascontiguousarray(b1, f32).reshape(1, C),
        "b2c": np.ascontiguousarray(b2, f32).reshape(C, 1),
        "b3c": np.ascontiguousarray(b3, f32).reshape(C, 1),
        "b4c": np.ascontiguousarray(b4, f32).reshape(C, 1),
        "bfr": np.ascontiguousarray(bf, f32).reshape(1, 7),
    }
    in_maps = []
    for k in range(NCORES):
        m = dict(base)
        sel = np.zeros((64, 5), f32)
        for c in range(5):
            sel[8 * k + c, c] = 1.0
        m["sel540"] = sel
        in_maps.append(m)
    return in_maps


def kernel(**inputs):
    from concourse.bass_utils import run_bass_kernel_spmd

    if "nc" not in _cache:
        _cache["nc"] = _build(debug=False)
    nc = _cache["nc"]
    in_maps = _prep_inputs(**inputs)
    res = run_bass_kernel_spmd(nc, in_maps, core_ids=list(range(NCORES)))
    out = np.asarray(res.results[0]["out"], dtype=np.float32)
    return out
